# revision 1
# baseline (speedup 1.0000x reference)
"""MoE routing gate kernel for Trainium2 (8 NeuronCores, data-parallel).

Problem (hardcoded): x [4, 4096, 2048] f32, w_gate [64, 2048] f32,
expert_bias [64] f32 (zeros per spec).
  gate_logits = x @ w_gate.T          # [B, S, 64]
  gate_weights = sigmoid(gate_logits)
  topk_vals, topk_idx = top_k(gate_logits + bias, k=8)
  topk_weights = gather(gate_weights, topk_idx); normalize
Returns (topk_weights [4,4096,8] f32, topk_indices [4,4096,8] int32).

Strategy: shard the 16384 tokens across 8 cores (2048 each); replicate
w_gate. Host pre-packs each core's x slice into a PE-friendly layout
[k, dp, g, tau] = x[token g*512+tau, d = k*128+dp], so the device
kernel streams large contiguous tiles from HBM straight into the
tensor engine's *moving* operand (the fast path for fp32) with the
small router weight as the stationary operand:
  psum_g[64 e, 512 tok] += wt_k[128 dp, 64 e].T @ x_k[128 dp, 512 tok]
accumulated over the 16 contraction chunks k into 4 PSUM banks.
Logits are then re-transposed token-major via 128x128 PE transposes,
and per 128-token tile the DVE max/max_index ops give the top-8
values+indices; ACT sigmoid (+row-sum), DVE reciprocal and scalar-mul
normalize. Expert bias is zeros per the problem spec, so biased
logits == logits (a numpy fallback guards the general case).
"""

import numpy as np

_B, _S, _D, _E = 4, 4096, 2048, 64
_K = 8
_NCORES = 8
_TOK = _B * _S              # 16384 tokens
_TC = _TOK // _NCORES       # 2048 tokens per core
_NG = 4                     # token groups of 512 per core
_GT = 512                   # tokens per group (PSUM bank / fp32 moving max)
_NKC = _D // 128            # 16 contraction chunks
_KGROUPS = (1, 1, 2, 4, 4, 4)   # k-chunks per DMA (graduated prefetch)

_prog_cache = {}


def _ensure_path():
    import sys
    for p in ("/opt/trn_rl_repo",):
        if p not in sys.path:
            sys.path.insert(0, p)


def _build_program(mode="f32"):
    """Per-core Bass/Tile program (SPMD: same program, different data)."""
    _ensure_path()
    import concourse.bass as bass
    import concourse.tile as tile
    from concourse import bacc, mybir

    nc = bacc.Bacc("TRN2", target_bir_lowering=False, debug=False,
                   num_devices=_NCORES)

    f32 = mybir.dt.float32
    u32 = mybir.dt.uint32
    mm_dt = mybir.dt.float32r if mode == "f32r" else f32

    # DRAM I/O (per core). x layout: [g, dp, k, tau] so each 512-token
    # group streams as one fully-contiguous-per-partition block.
    xg = nc.dram_tensor("xg", [_NG, 128, _NKC, _GT], mm_dt,
                        kind="ExternalInput")
    wt = nc.dram_tensor("wt", [128, _NKC * _E], mm_dt, kind="ExternalInput")
    ident = nc.dram_tensor("ident", [_E, _E], f32, kind="ExternalInput")
    out_w = nc.dram_tensor("out_w", [128, _NG, _NG, _K], f32,
                           kind="ExternalOutput")
    out_i = nc.dram_tensor("out_i", [128, _NG, _NG, _K], u32,
                           kind="ExternalOutput")

    # k-chunk split per group's DMA: fine-grained first loads so the PE
    # starts early; coarser afterwards for DMA efficiency.
    subchunks = ((1, 3, 4, 8), (8, 8), (8, 8), (8, 8))

    with tile.TileContext(nc) as tc:
        with (
            tc.tile_pool(name="xpool", bufs=2) as xpool,
            tc.tile_pool(name="wpool", bufs=1) as wpool,
            tc.tile_pool(name="psA", bufs=2, space=bass.MemorySpace.PSUM) as psA,
            tc.tile_pool(name="psB", bufs=2, space=bass.MemorySpace.PSUM) as psB,
            tc.tile_pool(name="lpool", bufs=2) as lpool,
            tc.tile_pool(name="opool", bufs=2) as opool,
            tc.tile_pool(name="tpool", bufs=4) as tpool,
        ):
            # All loads ride the sync ring in dependency order (FIFO per
            # ring; a side ring loses the SDMA round-robin to the bulk
            # stream). The k=0 weight slice goes first so the opening
            # matmul gates only on it plus the first 256 KiB x chunk.
            wt0_sb = wpool.tile([128, _E], mm_dt)
            nc.sync.dma_start(wt0_sb[:], wt[:, 0:_E])
            xt0 = xpool.tile([128, _NKC, _GT], mm_dt, tag="xg")
            nc.sync.dma_start(xt0[:, 0:1, :], xg[0][:, 0:1, :])
            wtR_sb = wpool.tile([128, (_NKC - 1) * _E], mm_dt)
            nc.sync.dma_start(wtR_sb[:], wt[:, _E:])
            id_sb = wpool.tile([_E, _E], f32)
            nc.sync.dma_start(id_sb[:], ident[:])

            def wt_k(k):
                return wt0_sb[:] if k == 0 else wtR_sb[:, bass.ts(k - 1, _E)]

            def postprocess(lg2, ig, wg, jg, nj):
                """top-8 + sigmoid + normalize for nj 128-token tiles."""
                for j in range(nj):
                    logit = lg2[:, j, :]
                    vals = tpool.tile([128, _K], f32, tag="vals")
                    nc.vector.max(vals[:], logit)
                    nc.vector.max_index(ig[:, jg + j, :], vals[:], logit)

                    sig = tpool.tile([128, _K], f32, tag="sig")
                    nc.scalar.activation(
                        sig[:], vals[:], mybir.ActivationFunctionType.Sigmoid,
                    )
                    ssum = tpool.tile([128, 1], f32, tag="ssum")
                    nc.vector.reduce_sum(
                        ssum[:], sig[:], axis=mybir.AxisListType.X,
                    )
                    rsum = tpool.tile([128, 1], f32, tag="rsum")
                    nc.vector.reciprocal(rsum[:], ssum[:])
                    nc.vector.tensor_scalar_mul(wg[:, jg + j, :], sig[:], rsum[:])

            def transpose_block(ps, toff, ntok, ig, wg):
                """logitsT psum [64, ntok] -> token-major topk results."""
                nj = ntok // 128
                lg = lpool.tile([_E, _GT], f32, tag="lg")
                nc.scalar.copy(lg[:, :ntok], ps[:])
                ps2 = psB.tile([128, _NG, _E], f32, tag="ps2")
                for j in range(nj):
                    nc.tensor.transpose(
                        ps2[:, j, :], lg[:, bass.ts(j, 128)], id_sb[:],
                    )
                lg2 = lpool.tile([128, _NG, _E], f32, tag="lg2")
                nc.scalar.copy(lg2[:, :nj, :], ps2[:, :nj, :])
                postprocess(lg2, ig, wg, toff // 128, nj)

            for g in range(_NG):
                xt = xt0 if g == 0 else xpool.tile([128, _NKC, _GT], mm_dt,
                                                   tag="xg")
                k0 = 1 if g == 0 else 0
                for nk in subchunks[g]:
                    nk = min(nk, _NKC - k0)
                    nc.sync.dma_start(
                        xt[:, k0:k0 + nk, :],
                        xg[g][:, k0:k0 + nk, :],
                    )
                    k0 += nk

                # Last group: two 256-token accumulators so the final
                # top-k tail is halved (first half overlaps second's MMs).
                splits = ((0, 512),) if g < _NG - 1 else ((0, 256), (256, 256))
                pss = []
                for toff, ntok in splits:
                    ps = psA.tile([_E, ntok], f32, tag=f"ps{len(pss)}")
                    pss.append(ps)
                    for k in range(_NKC):
                        nc.tensor.matmul(
                            ps[:], wt_k(k),
                            xt[:, k, toff:toff + ntok],
                            start=(k == 0), stop=(k == _NKC - 1),
                        )

                wg = opool.tile([128, _NG, _K], f32, tag="wg")
                ig = opool.tile([128, _NG, _K], u32, tag="ig")
                for (toff, ntok), ps in zip(splits, pss):
                    transpose_block(ps, toff, ntok, ig, wg)
                    j0, j1 = toff // 128, (toff + ntok) // 128
                    nc.scalar.dma_start(out_w[:, g, j0:j1], wg[:, j0:j1])
                    nc.scalar.dma_start(out_i[:, g, j0:j1], ig[:, j0:j1])

    nc.compile()
    return nc


def _get_program(mode="f32"):
    if mode not in _prog_cache:
        _prog_cache[mode] = _build_program(mode)
    return _prog_cache[mode]


def _pack_inputs(x, w_gate):
    """Host-side layout transform. Returns per-core input maps."""
    x2 = np.ascontiguousarray(x, dtype=np.float32).reshape(_TOK, _D)
    # wt[dp, k*64+e] = w_gate[e, k*128+dp]
    wt = np.ascontiguousarray(
        w_gate.T.reshape(_NKC, 128, _E).transpose(1, 0, 2).reshape(128, _NKC * _E),
        dtype=np.float32,
    )
    ident = np.eye(_E, dtype=np.float32)
    in_maps = []
    for c in range(_NCORES):
        xc = x2[c * _TC:(c + 1) * _TC]                 # [2048 tok, 2048 d]
        # [g, tau, k, dp] -> [g, dp, k, tau]
        xgc = np.ascontiguousarray(
            xc.reshape(_NG, _GT, _NKC, 128).transpose(0, 3, 2, 1)
        )
        in_maps.append({"xg": xgc, "wt": wt, "ident": ident})
    return in_maps


def _unpack_outputs(results):
    w_parts, i_parts = [], []
    for r in results:
        # [128 tau, 4 g, 4 j, 8] -> token (4g+j)*128+tau -> [2048, 8]
        w_parts.append(
            r["out_w"].reshape(128, _NG * _NG, _K).transpose(1, 0, 2).reshape(_TC, _K)
        )
        i_parts.append(
            r["out_i"].reshape(128, _NG * _NG, _K).transpose(1, 0, 2).reshape(_TC, _K)
        )
    weights = np.concatenate(w_parts, axis=0).reshape(_B, _S, _K)
    indices = (
        np.concatenate(i_parts, axis=0).astype(np.int32).reshape(_B, _S, _K)
    )
    return weights, indices


def _numpy_reference(x, w_gate, expert_bias):
    """Exact fallback for the (unspecced) nonzero-bias case."""
    x2 = np.asarray(x, dtype=np.float32).reshape(_TOK, _D)
    logits = x2 @ np.asarray(w_gate, dtype=np.float32).T
    gw = 1.0 / (1.0 + np.exp(-logits))
    biased = logits + np.asarray(expert_bias, dtype=np.float32)
    idx = np.argsort(-biased, axis=-1, kind="stable")[:, :_K].astype(np.int32)
    tw = np.take_along_axis(gw, idx, axis=-1)
    tw = tw / tw.sum(axis=-1, keepdims=True)
    return (
        tw.reshape(_B, _S, _K).astype(np.float32),
        idx.reshape(_B, _S, _K).astype(np.int32),
    )


def _run(x, w_gate, expert_bias, trace=False, mode="f32", trace_kwargs=None):
    _ensure_path()
    from concourse.bass_utils import run_bass_kernel_spmd

    nc = _get_program(mode)
    in_maps = _pack_inputs(x, w_gate)
    res = run_bass_kernel_spmd(
        nc, in_maps, list(range(_NCORES)), trace=trace,
        **(trace_kwargs or {}),
    )
    weights, indices = _unpack_outputs(res.results)
    return (weights, indices), res


def kernel(x, w_gate, expert_bias):
    x = np.asarray(x)
    w_gate = np.asarray(w_gate)
    expert_bias = np.asarray(expert_bias)
    assert x.shape == (_B, _S, _D), x.shape
    assert w_gate.shape == (_E, _D), w_gate.shape
    if np.any(expert_bias):
        # Spec pins expert_bias to zeros; keep a correct host path anyway.
        return _numpy_reference(x, w_gate, expert_bias)
    try:
        (weights, indices), _ = _run(x, w_gate, expert_bias)
    except Exception:
        # Transient NRT device wedges have been observed on a first
        # execution; one retry has always recovered.
        import time
        time.sleep(10)
        (weights, indices), _ = _run(x, w_gate, expert_bias)
    return weights, indices

